# revision 1
# baseline (speedup 1.0000x reference)
"""Trainium2 Bass kernel for nn_NewCNNEncoder (dense CNN encoder over one-hot boards).

Strategy (pure data parallel over 8 NeuronCores, 8192 samples each):
  - The input x [B, 25] (values 0..16) is one-hot encoded ON CHIP via
    broadcast-matmul + is_equal compare, in three layouts matched to the
    three depthwise-conv branches (full / horizontal / vertical).
  - All convolutions are expressed as dense matmuls with activations kept
    in [features-on-partitions, batch-free] layout; the final conv_out
    layer flips to [batch-on-partitions, features-free] so the output DMA
    is contiguous.
  - Matmuls run in float32r (full-rate fp32 mode) except the first-layer
    broadcast and the last layer, which run in bf16.
  - leaky_relu(+bias) epilogues are single ScalarE activation ops reading
    PSUM directly.
"""

import sys

sys.path.insert(0, "/opt/trn_rl_repo")

import numpy as np
import ml_dtypes

import concourse.mybir as mybir
import concourse.tile as tile
from concourse import bacc
from concourse.bass_utils import run_bass_kernel_spmd

NCORES = 8
B_FULL = 65536
BC = B_FULL // NCORES  # 8192 per core
NT = 512               # batch tile (samples per pipeline tile)
NTILES = BC // NT      # 16

NC_ = 25   # cells (5x5 board)
NCL = 17   # classes
MULT = 16
OC = 64
OF = 1600
SLOPE = 0.01

F32 = mybir.dt.float32
F32R = mybir.dt.float32r
BF16 = mybir.dt.bfloat16
BF16NP = ml_dtypes.bfloat16
LRELU = mybir.ActivationFunctionType.Lrelu
EQ = mybir.AluOpType.is_equal

# one-hot row chunking, 425 rows (p = 25c + l) zero-padded to 512
FULL_CH = [(0, 128), (128, 128), (256, 128), (384, 128)]
# per-r (and per-j) L1 output col chunks, 272 (16c + m) zero-padded to 384
H_CH = [(0, 128), (128, 128), (256, 128)]
# L2-full output chunks over 320
F2_CH = [(0, 128), (128, 128), (256, 64)]
# act2 (cat) K-chunk sizes: 7x128 + 65 (last = vert j4 (64) + ones row)
A2_SIZES = [128] * 7 + [65]
# where each L2 output block lands in the A2 tiles: branch -> (tile, part_off)
H_DST = {0: (2, 64), 1: (3, 0), 2: (3, 64), 3: (4, 0), 4: (4, 64)}
V_DST = {0: (5, 0), 1: (5, 64), 2: (6, 0), 3: (6, 64), 4: (7, 0)}


def _fr(c, l):
    """class-aligned padded row/feature index for (class, cell)"""
    return 128 * (c // 5) + 25 * (c % 5) + l


def _build_nc():
    nc = bacc.Bacc("TRN2", target_bir_lowering=False, debug=False)

    # ---- DRAM I/O ----
    d_xt = nc.dram_tensor("xt", [128, BC], BF16, kind="ExternalInput")
    d_sf = nc.dram_tensor("sf", [128, 512], BF16, kind="ExternalInput")
    d_sh = nc.dram_tensor("sh", [128, 425], BF16, kind="ExternalInput")
    d_sv = nc.dram_tensor("sv", [128, 425], BF16, kind="ExternalInput")
    d_clsf = nc.dram_tensor("clsf", [128, 4], F32, kind="ExternalInput")
    d_clsh = nc.dram_tensor("clsh", [85, 1], F32, kind="ExternalInput")
    d_a1f = nc.dram_tensor("a1f", [512, 512], F32R, kind="ExternalInput")
    d_a1h = nc.dram_tensor("a1h", [85, 384], F32R, kind="ExternalInput")
    d_a1v = nc.dram_tensor("a1v", [85, 384], F32R, kind="ExternalInput")
    d_w2f = nc.dram_tensor("w2f", [512, 320], F32R, kind="ExternalInput")
    d_w2h = nc.dram_tensor("w2h", [384, 64], F32R, kind="ExternalInput")
    d_w2v = nc.dram_tensor("w2v", [384, 64], F32R, kind="ExternalInput")
    d_w3 = nc.dram_tensor("w3", [961, OF], BF16, kind="ExternalInput")
    d_b1f = nc.dram_tensor("b1f", [128, 4], F32, kind="ExternalInput")
    d_b1h = nc.dram_tensor("b1h", [128, 3], F32, kind="ExternalInput")
    d_b1v = nc.dram_tensor("b1v", [128, 3], F32, kind="ExternalInput")
    d_b2f = nc.dram_tensor("b2f", [128, 3], F32, kind="ExternalInput")
    d_b2h = nc.dram_tensor("b2h", [64, 1], F32, kind="ExternalInput")
    d_b2v = nc.dram_tensor("b2v", [64, 1], F32, kind="ExternalInput")
    d_y = nc.dram_tensor("y", [BC, OF], F32, kind="ExternalOutput")

    with tile.TileContext(nc) as tc:
        with (
            tc.tile_pool(name="const", bufs=1) as cp,
            tc.tile_pool(name="work", bufs=2) as wp,
            tc.tile_pool(name="oh", bufs=3) as ohp,
            tc.tile_pool(name="outp", bufs=3) as op_,
            tc.tile_pool(name="ps_s", bufs=4, space="PSUM") as pp,
            tc.tile_pool(name="ps_l3", bufs=2, space="PSUM") as pp3,
        ):
            # ---- load constants/weights into SBUF ----
            xt = cp.tile([128, BC], BF16, tag="xt")
            for t_i in range(NTILES):
                nc.sync.dma_start(xt[:, t_i * NT:(t_i + 1) * NT],
                                  d_xt[:, t_i * NT:(t_i + 1) * NT])
            sf = cp.tile([128, 512], BF16, tag="sf")
            nc.sync.dma_start(sf[:], d_sf[:])
            sh = cp.tile([128, 425], BF16, tag="sh")
            nc.sync.dma_start(sh[:], d_sh[:])
            sv = cp.tile([128, 425], BF16, tag="sv")
            nc.sync.dma_start(sv[:], d_sv[:])
            clsf = cp.tile([128, 4], F32, tag="clsf")
            nc.sync.dma_start(clsf[:], d_clsf[:])
            clsh = cp.tile([85, 1], F32, tag="clsh")
            nc.sync.dma_start(clsh[:], d_clsh[:])

            a1f = []
            for kc, (k0, kp) in enumerate(FULL_CH):
                t = cp.tile([kp, 512], F32R, tag=f"a1f_{kc}")
                nc.sync.dma_start(t[:], d_a1f[k0:k0 + kp, :])
                a1f.append(t)
            a1h = cp.tile([85, 384], F32R, tag="a1h")
            nc.sync.dma_start(a1h[:], d_a1h[:])
            a1v = cp.tile([85, 384], F32R, tag="a1v")
            nc.sync.dma_start(a1v[:], d_a1v[:])

            w2f = []
            for kc, (k0, kp) in enumerate(FULL_CH):
                t = cp.tile([kp, 320], F32R, tag=f"w2f_{kc}")
                nc.sync.dma_start(t[:], d_w2f[k0:k0 + kp, :])
                w2f.append(t)
            w2h = []
            w2v = []
            for kc, (k0, kp) in enumerate(H_CH):
                t = cp.tile([kp, 64], F32R, tag=f"w2h_{kc}")
                nc.sync.dma_start(t[:], d_w2h[k0:k0 + kp, :])
                w2h.append(t)
                t = cp.tile([kp, 64], F32R, tag=f"w2v_{kc}")
                nc.sync.dma_start(t[:], d_w2v[k0:k0 + kp, :])
                w2v.append(t)
            w3 = []
            r0 = 0
            for i, sz in enumerate(A2_SIZES):
                t = cp.tile([sz, OF], BF16, tag=f"w3_{i}")
                nc.sync.dma_start(t[:], d_w3[r0:r0 + sz, :])
                w3.append(t)
                r0 += sz

            b1f = cp.tile([128, 4], F32, tag="b1f")
            nc.sync.dma_start(b1f[:], d_b1f[:])
            b1h = cp.tile([128, 3], F32, tag="b1h")
            nc.sync.dma_start(b1h[:], d_b1h[:])
            b1v = cp.tile([128, 3], F32, tag="b1v")
            nc.sync.dma_start(b1v[:], d_b1v[:])
            b2f = cp.tile([128, 3], F32, tag="b2f")
            nc.sync.dma_start(b2f[:], d_b2f[:])
            b2h = cp.tile([64, 1], F32, tag="b2h")
            nc.sync.dma_start(b2h[:], d_b2h[:])
            b2v = cp.tile([64, 1], F32, tag="b2v")
            nc.sync.dma_start(b2v[:], d_b2v[:])

            # ---- batch-tile pipeline ----
            for t_i in range(NTILES):
                n0 = t_i * NT
                xs = xt[:, n0:n0 + NT]

                A2 = [wp.tile([A2_SIZES[i], NT], BF16, tag=f"a2_{i}",
                              name=f"a2_{i}_{t_i}")
                      for i in range(8)]

                # ===== full branch =====
                ohf = []
                for kc, (k0, kp) in enumerate(FULL_CH):
                    ps = pp.tile([kp, NT], F32, tag="ps_s")
                    nc.tensor.matmul(ps[:], sf[:, k0:k0 + kp], xs,
                                     start=True, stop=True)
                    oht = ohp.tile([kp, NT], F32R, tag=f"ohf{kc}")
                    nc.vector.tensor_scalar(oht[:], ps[:],
                                            clsf[0:kp, kc:kc + 1], None,
                                            op0=EQ)
                    ohf.append(oht)

                act1f = []
                for mc, (m0, mp) in enumerate(FULL_CH):
                    ps = pp.tile([mp, NT], F32, tag="ps_s")
                    nc.tensor.matmul(ps[:], a1f[mc][:, m0:m0 + mp],
                                     ohf[mc][:], start=True, stop=True)
                    a = wp.tile([mp, NT], F32R, tag=f"act1f{mc}")
                    nc.scalar.activation(a[:], ps[:], LRELU,
                                         bias=b1f[0:mp, mc:mc + 1],
                                         alpha=SLOPE)
                    act1f.append(a)

                for mc2, (m0, mp) in enumerate(F2_CH):
                    ps = pp.tile([mp, NT], F32, tag="ps_s")
                    for i in range(4):
                        nc.tensor.matmul(ps[:], w2f[i][:, m0:m0 + mp],
                                         act1f[i][:],
                                         start=(i == 0), stop=(i == 3))
                    if mc2 < 2:
                        dst = A2[mc2][0:128, :]
                    else:
                        dst = A2[2][0:64, :]
                    nc.scalar.activation(dst, ps[:], LRELU,
                                         bias=b2f[0:mp, mc2:mc2 + 1],
                                         alpha=SLOPE)

                # ===== hori / vert branches: stage-offset software pipeline =====
                BR = {
                    "h": (sh, a1h, w2h, b1h, b2h, H_DST),
                    "v": (sv, a1v, w2v, b1v, b2v, V_DST),
                }
                pairs = [("h", r) for r in range(5)] + \
                        [("v", r) for r in range(5)]
                oh_l, a1_l = {}, {}

                def _gen(br, r):
                    s_mat = BR[br][0]
                    ps = pp.tile([85, NT], F32, tag="ps_s",
                                 name=f"psb_{br}{r}_{t_i}")
                    nc.tensor.matmul(ps[:], s_mat[:, 85 * r:85 * r + 85],
                                     xs, start=True, stop=True)
                    ohr = ohp.tile([85, NT], F32R, tag=f"oh{br}",
                                   name=f"oh{br}{r}_{t_i}")
                    nc.vector.tensor_scalar(ohr[:], ps[:], clsh[:, 0:1],
                                            None, op0=EQ)
                    oh_l[(br, r)] = ohr

                def _l1(br, r):
                    a1_mat, b1_t = BR[br][1], BR[br][3]
                    ohr = oh_l.pop((br, r))
                    ts = []
                    for mc, (m0, mp) in enumerate(H_CH):
                        ps1 = pp.tile([mp, NT], F32, tag="ps_s",
                                      name=f"ps1_{br}{r}{mc}_{t_i}")
                        nc.tensor.matmul(ps1[:], a1_mat[:, m0:m0 + mp],
                                         ohr[:], start=True, stop=True)
                        a = wp.tile([mp, NT], F32R, tag=f"act1{br}{mc}",
                                    name=f"act1{br}{r}{mc}_{t_i}")
                        nc.scalar.activation(a[:], ps1[:], LRELU,
                                             bias=b1_t[0:mp, mc:mc + 1],
                                             alpha=SLOPE)
                        ts.append(a)
                    a1_l[(br, r)] = ts

                def _l2(br, r):
                    w2_t, b2_t, dst_map = BR[br][2], BR[br][4], BR[br][5]
                    a1_t = a1_l.pop((br, r))
                    ps2 = pp.tile([64, NT], F32, tag="ps_s",
                                  name=f"ps2_{br}{r}_{t_i}")
                    for i, (m0, mp) in enumerate(H_CH):
                        nc.tensor.matmul(ps2[:], w2_t[i][:, 0:64],
                                         a1_t[i][:],
                                         start=(i == 0), stop=(i == 2))
                    ti, off = dst_map[r]
                    nc.scalar.activation(A2[ti][off:off + 64, :], ps2[:],
                                         LRELU, bias=b2_t[0:64, 0:1],
                                         alpha=SLOPE)

                for idx in range(len(pairs) + 2):
                    if idx < len(pairs):
                        _gen(*pairs[idx])
                    if 0 <= idx - 1 < len(pairs):
                        _l1(*pairs[idx - 1])
                    if 0 <= idx - 2 < len(pairs):
                        _l2(*pairs[idx - 2])

                # ones row for the bias of the output layer
                nc.vector.memset(A2[7][64:65, :], 1.0)

                # ===== output layer (batch on partitions) =====
                for q in range(4):
                    b0 = q * 128
                    o = op_.tile([128, OF], F32, tag="outt")
                    for half in range(2):
                        c0 = half * 800
                        ps3 = pp3.tile([128, 800], F32, tag="ps_l3",
                                       name=f"ps3_{q}{half}_{t_i}")
                        for i in range(8):
                            lh = A2[i][:, b0:b0 + 128]
                            st, sp_ = (i == 0), (i == 7)
                            nc.tensor.matmul(ps3[:, 0:512], lh,
                                             w3[i][:, c0:c0 + 512],
                                             start=st, stop=sp_)
                            nc.tensor.matmul(ps3[:, 512:800], lh,
                                             w3[i][:, c0 + 512:c0 + 800],
                                             start=st, stop=sp_)
                        nc.scalar.activation(o[:, c0:c0 + 800], ps3[:],
                                             LRELU, alpha=SLOPE)
                    nc.sync.dma_start(d_y[n0 + b0:n0 + b0 + 128, :], o[:])

    nc.compile()
    return nc


_NC_CACHE = None


def _get_nc():
    global _NC_CACHE
    if _NC_CACHE is None:
        _NC_CACHE = _build_nc()
    return _NC_CACHE


def _prep_weights(inputs):
    W_df = np.asarray(inputs["W_df"], dtype=np.float32)
    b_df = np.asarray(inputs["b_df"], dtype=np.float32)
    W_pf = np.asarray(inputs["W_pf"], dtype=np.float32)
    b_pf = np.asarray(inputs["b_pf"], dtype=np.float32)
    W_dh = np.asarray(inputs["W_dh"], dtype=np.float32)
    b_dh = np.asarray(inputs["b_dh"], dtype=np.float32)
    W_ph = np.asarray(inputs["W_ph"], dtype=np.float32)
    b_ph = np.asarray(inputs["b_ph"], dtype=np.float32)
    W_dv = np.asarray(inputs["W_dv"], dtype=np.float32)
    b_dv = np.asarray(inputs["b_dv"], dtype=np.float32)
    W_pv = np.asarray(inputs["W_pv"], dtype=np.float32)
    b_pv = np.asarray(inputs["b_pv"], dtype=np.float32)
    W_out = np.asarray(inputs["W_out"], dtype=np.float32)
    b_out = np.asarray(inputs["b_out"], dtype=np.float32)

    cc = np.arange(NCL)
    ll = np.arange(NC_)

    A_full = np.zeros((512, 512), np.float32)
    for c in range(NCL):
        r0, c0 = _fr(c, 0), _fr(c, 0)
        # block [l, m] = W_df[c, m, l]
        A_full[r0:r0 + 25, c0:c0 + 25] = W_df[c].T
    A_h = np.zeros((85, 384), np.float32)
    A_v = np.zeros((85, 384), np.float32)
    for c in range(NCL):
        A_h[5 * c:5 * c + 5, 16 * c:16 * c + 16] = W_dh[c].T  # [j, m]
        A_v[5 * c:5 * c + 5, 16 * c:16 * c + 16] = W_dv[c].T  # [r, m]

    # selection (broadcast) matrices, bf16-exact 0/1 (K padded 25 -> 128)
    sf = np.zeros((128, 512), BF16NP)
    for c in range(NCL):
        for l in range(NC_):
            sf[l, _fr(c, l)] = 1
    # sh: col 85*r + 5*c + j -> row l = 5*r + j
    sh = np.zeros((128, 425), BF16NP)
    # sv: col 85*j + 5*c + r -> row l = 5*r + j
    sv = np.zeros((128, 425), BF16NP)
    for c in range(NCL):
        for r in range(5):
            for j in range(5):
                sh[5 * r + j, 85 * r + 5 * c + j] = 1
                sv[5 * r + j, 85 * j + 5 * c + r] = 1

    # class constant per one-hot row; -1 on padding rows (matches nothing)
    clsf = np.full((128, 4), -1.0, np.float32)
    for kc in range(4):
        for c in range(5 * kc, min(5 * kc + 5, NCL)):
            p0 = 25 * (c % 5)
            clsf[p0:p0 + 25, kc] = float(c)
    clsh = (np.arange(85) // 5).astype(np.float32)[:, None]

    # output-layer weights, rows reordered to the act2 chunk layout
    W3re = np.zeros((961, OF), np.float32)
    W3re[0:320] = W_out[:, :, 0:5].transpose(1, 2, 0).reshape(320, OF)
    W3re[320:640] = W_out[:, :, 5:10].transpose(2, 1, 0).reshape(320, OF)
    W3re[640:960] = W_out[:, :, 10:15].transpose(2, 1, 0).reshape(320, OF)
    W3re[960] = b_out

    b1f = np.zeros((128, 4), np.float32)
    for mc in range(4):
        for c in range(5 * mc, min(5 * mc + 5, NCL)):
            p0 = 25 * (c % 5)
            b1f[p0:p0 + 25, mc] = b_df[25 * c:25 * c + 25]
    b1h = np.zeros((128, 3), np.float32)
    b1v = np.zeros((128, 3), np.float32)
    for mc, (m0, mp) in enumerate(H_CH):
        valid = max(0, min(mp, 272 - m0))
        b1h[0:valid, mc] = b_dh[m0:m0 + valid]
        b1v[0:valid, mc] = b_dv[m0:m0 + valid]
    b2f = np.zeros((128, 3), np.float32)
    for mc, (m0, mp) in enumerate(F2_CH):
        b2f[0:mp, mc] = b_pf[m0:m0 + mp]

    w2f_p = np.zeros((512, 320), np.float32)
    for c in range(NCL):
        r0 = _fr(c, 0)
        w2f_p[r0:r0 + 25] = W_pf.T[25 * c:25 * c + 25]
    w2h_p = np.zeros((384, 64), np.float32)
    w2h_p[0:272] = W_ph.T
    w2v_p = np.zeros((384, 64), np.float32)
    w2v_p[0:272] = W_pv.T

    return {
        "sf": sf, "sh": sh, "sv": sv,
        "clsf": clsf, "clsh": clsh,
        "a1f": A_full, "a1h": A_h, "a1v": A_v,
        "w2f": w2f_p, "w2h": w2h_p, "w2v": w2v_p,
        "w3": W3re.astype(BF16NP),
        "b1f": b1f, "b1h": b1h, "b1v": b1v,
        "b2f": b2f,
        "b2h": b_ph[:, None].copy(),
        "b2v": b_pv[:, None].copy(),
    }


def kernel(**inputs) -> np.ndarray:
    x = np.asarray(inputs["x"]).astype(np.int32)
    assert x.shape == (B_FULL, NC_), x.shape

    shared = _prep_weights(inputs)
    nc = _get_nc()

    in_maps = []
    for core in range(NCORES):
        xs = x[core * BC:(core + 1) * BC]          # [BC, 25]
        xtc = np.zeros((128, BC), BF16NP)
        xtc[:NC_] = xs.T.astype(BF16NP)
        m = dict(shared)
        m["xt"] = xtc
        in_maps.append(m)

    res = run_bass_kernel_spmd(nc, in_maps, core_ids=list(range(NCORES)))
    global LAST_RESULTS
    LAST_RESULTS = res
    out = np.concatenate([res.results[i]["y"] for i in range(NCORES)], axis=0)
    return out


LAST_RESULTS = None



# revision 17
# speedup vs baseline: 1.3094x; 1.3094x over previous
"""Trainium2 Bass kernel for nn_NewCNNEncoder (dense CNN encoder over one-hot boards).

Strategy (pure data parallel over 8 NeuronCores, 8192 samples each):
  - One-hot encodings of x are built on the host in three layouts (full /
    horizontal / vertical), stored as exact fp8(e4m3) 0/1 bytes and DMA'd in.
    L1 bias rows ride in one-hot pad rows (ones), so L1 activations are
    bias-free; weights are bf16.
  - All matmuls are bf16 (PE streams 1 column/cycle regardless of dtype; the
    real cost is sum of instruction N, so the win is minimizing instruction
    count x N and keeping K >= 64 everywhere - small-K matmuls run ~1.8x
    slower).
  - The h/v L1 runt columns (features 256:272 of each slice) are merged into
    ONE fp8 DoubleRow matmul per slice pair: plane0 carries the h-runt into
    M columns 0:16, plane1 the v-runt into 32:48 (disjoint M via zero
    columns), contracting different one-hot slices in one instruction.
  - The output conv (K=961 -> 8 chunks of 128, N=1600) runs in bf16 as
    2x(512+288) psum halves so LDWEIGHTS stays hidden.
  - Activations: single-pass bf16 lrelu on Act; the final lrelu runs on DVE
    (mul+max) into bf16 outputs.
  - The output layer of tile t-1 is woven through tile t's emission so the
    PE never idles waiting on Act/DVE psum drains.
"""

import sys

sys.path.insert(0, "/opt/trn_rl_repo")

import numpy as np
import ml_dtypes

import concourse.mybir as mybir
import concourse.tile as tile
from concourse import bacc
from concourse.bass_utils import run_bass_kernel_spmd

NCORES = 8
B_FULL = 65536
BC = B_FULL // NCORES  # 8192
NT = 512
NTILES = BC // NT      # 16

F32 = mybir.dt.float32
F8 = mybir.dt.float8e4
BF16 = mybir.dt.bfloat16
E4 = ml_dtypes.float8_e4m3
BF = ml_dtypes.bfloat16
DR = mybir.MatmulPerfMode.DoubleRow
LRELU = mybir.ActivationFunctionType.Lrelu
MULT = mybir.AluOpType.mult
MAX = mybir.AluOpType.max
SLOPE = 0.01

SW1R = 8.0  # fp8 scale for the merged-runt L1 weights


def _build_nc():
    nc = bacc.Bacc("TRN2", target_bir_lowering=False, debug=False)

    d_ohf = nc.dram_tensor("ohf", [128, NTILES * 2048], F8, kind="ExternalInput")
    d_ohhv = nc.dram_tensor("ohhv", [86, NTILES * 5120], F8, kind="ExternalInput")
    d_w1f = nc.dram_tensor("w1f", [128, 512], BF16, kind="ExternalInput")
    d_w1h = nc.dram_tensor("w1h", [86, 256], BF16, kind="ExternalInput")
    d_w1v = nc.dram_tensor("w1v", [86, 256], BF16, kind="ExternalInput")
    d_w1r = nc.dram_tensor("w1r", [86, 128], F8, kind="ExternalInput")
    d_w2f = nc.dram_tensor("w2f", [128, 1536], BF16, kind="ExternalInput")
    d_w2h = nc.dram_tensor("w2h", [128, 128], BF16, kind="ExternalInput")
    d_w2v = nc.dram_tensor("w2v", [128, 128], BF16, kind="ExternalInput")
    d_w2r = nc.dram_tensor("w2r", [64, 128], BF16, kind="ExternalInput")
    d_w3 = nc.dram_tensor("w3", [128, 12800], BF16, kind="ExternalInput")
    d_b2f = nc.dram_tensor("b2f", [128, 3], F32, kind="ExternalInput")
    d_b2hv = nc.dram_tensor("b2hv", [128, 1], F32, kind="ExternalInput")
    d_y = nc.dram_tensor("y", [BC, 1600], BF16, kind="ExternalOutput")

    with tile.TileContext(nc) as tc:
        with (
            tc.tile_pool(name="const", bufs=1) as cp,
            tc.tile_pool(name="ohp", bufs=2) as ohp,
            tc.tile_pool(name="a1p", bufs=2) as a1p,
            tc.tile_pool(name="a2p", bufs=2) as a2p,
            tc.tile_pool(name="yp", bufs=3) as yp,
            tc.tile_pool(name="ps", bufs=2, space="PSUM") as pp,
            tc.tile_pool(name="ps3", bufs=2, space="PSUM") as pp3,
        ):
            # ---- load weights (w3 last so tile0 compute starts early) ----
            w1f = cp.tile([128, 4, 128], BF16, tag="w1f")
            nc.sync.dma_start(w1f[:], d_w1f[:, :])
            w1h = cp.tile([86, 2, 128], BF16, tag="w1h")
            nc.sync.dma_start(w1h[:], d_w1h[:, :])
            w1v = cp.tile([86, 2, 128], BF16, tag="w1v")
            nc.sync.dma_start(w1v[:], d_w1v[:, :])
            w1r = cp.tile([86, 2, 64], F8, tag="w1r")
            nc.sync.dma_start(w1r[:], d_w1r[:, :])
            w2f = cp.tile([128, 4, 384], BF16, tag="w2f")
            nc.sync.dma_start(w2f[:], d_w2f[:, :])
            w2h = cp.tile([128, 2, 64], BF16, tag="w2h")
            nc.sync.dma_start(w2h[:], d_w2h[:, :])
            w2v = cp.tile([128, 2, 64], BF16, tag="w2v")
            nc.sync.dma_start(w2v[:], d_w2v[:, :])
            w2r = cp.tile([64, 128], BF16, tag="w2r")
            nc.sync.dma_start(w2r[:], d_w2r[:, :])
            b2f = cp.tile([128, 3], F32, tag="b2f")
            nc.sync.dma_start(b2f[:], d_b2f[:, :])
            b2hv = cp.tile([128, 1], F32, tag="b2hv")
            nc.sync.dma_start(b2hv[:], d_b2hv[:, :])
            w3 = cp.tile([128, 8, 1600], BF16, tag="w3")
            for k in range(8):
                nc.sync.dma_start(w3[:, k, :], d_w3[:, k * 1600:(k + 1) * 1600])

            prev = None  # (a2, t) of previous tile

            def out_groups(a2_p, t_p):
                """Generator: one (bchunk, half) out-layer group per next()."""
                for b in range(4):
                    yt = yp.tile([128, 1600], BF16, tag="y",
                                 name=f"y_{t_p}_{b}")
                    for h in range(2):
                        c0 = 800 * h
                        ps = pp3.tile([128, 1024], F32, tag="ps3",
                                      name=f"ps3_{t_p}_{b}_{h}")
                        for q in range(8):
                            nc.tensor.matmul(
                                ps[:, 0:512],
                                a2_p[:, q, 128 * b:128 * b + 128],
                                w3[:, q, c0:c0 + 512],
                                start=(q == 0), stop=(q == 7))
                            nc.tensor.matmul(
                                ps[:, 512:800],
                                a2_p[:, q, 128 * b:128 * b + 128],
                                w3[:, q, c0 + 512:c0 + 800],
                                start=(q == 0), stop=(q == 7))
                        ytmp = yp.tile([128, 800], BF16, tag="ytmp",
                                       bufs=2, name=f"ytmp_{t_p}_{b}_{h}")
                        nc.vector.tensor_scalar(ytmp[:], ps[:, 0:800],
                                                SLOPE, None, op0=MULT)
                        nc.vector.tensor_tensor(yt[:, c0:c0 + 800],
                                                ps[:, 0:800],
                                                ytmp[:], op=MAX)
                        yield
                    nc.gpsimd.dma_start(
                        d_y[t_p * NT + 128 * b: t_p * NT + 128 * b + 128, :],
                        yt[:])
                while True:
                    yield

            def weave(gen, n):
                if gen is not None:
                    for _ in range(n):
                        next(gen)

            for t in range(NTILES):
                ohf = ohp.tile([128, 4, 512], F8, tag="ohf", name=f"ohf_{t}")
                nc.sync.dma_start(ohf[:], d_ohf[:, t * 2048:(t + 1) * 2048])
                ohhv = ohp.tile([86, 5, 2, 512], F8, tag="ohhv",
                                name=f"ohhv_{t}")
                nc.sync.dma_start(ohhv[:], d_ohhv[:, t * 5120:(t + 1) * 5120])

                og = out_groups(*prev) if prev is not None else None

                a1f = a1p.tile([128, 4, 512], BF16, tag="a1f", name=f"a1f_{t}")
                a1h = a1p.tile([128, 5, 2, 512], BF16, tag="a1h", name=f"a1h_{t}")
                a1v = a1p.tile([128, 5, 2, 512], BF16, tag="a1v", name=f"a1v_{t}")
                a1r = a1p.tile([128, 5, 512], BF16, tag="a1r", name=f"a1r_{t}")
                a2 = a2p.tile([128, 8, 512], BF16, tag="a2", name=f"a2_{t}")

                # ===== L1 full (block-diag per chunk, bias in pad row) =====
                for g in range(2):
                    ps = pp.tile([128, 2, 512], F32, tag="ps", name=f"psf_{t}_{g}")
                    for kk in range(2):
                        k = 2 * g + kk
                        nc.tensor.matmul(ps[:, kk, :],
                                         w1f[:, k, :], ohf[:, k, :],
                                         start=True, stop=True)
                    nc.scalar.activation(a1f[:, 2 * g:2 * g + 2, :], ps[:],
                                         LRELU, alpha=SLOPE)
                weave(og, 1)

                # ===== L1 hori/vert (5 slice pairs + merged DR runt) =====
                for r in range(5):
                    for i, (w1b, a1b) in enumerate(((w1h, a1h), (w1v, a1v))):
                        ps = pp.tile([128, 2, 512], F32, tag="ps",
                                     name=f"ps1_{t}_{r}_{i}")
                        for m in range(2):
                            nc.tensor.matmul(ps[:, m, :],
                                             w1b[:, m, :],
                                             ohhv[:, r, i, :],
                                             start=True, stop=True)
                        nc.scalar.activation(a1b[:, r, :, :], ps[:], LRELU,
                                             alpha=SLOPE)
                    prt = pp.tile([128, 512], F32, tag="ps", name=f"psrt_{t}_{r}")
                    nc.tensor.matmul(prt[0:64, :], w1r[:], ohhv[:, r, :, :],
                                     start=True, stop=True, perf_mode=DR)
                    nc.scalar.activation(a1r[0:64, r, :], prt[0:64, :], LRELU,
                                         scale=1.0 / SW1R, alpha=SLOPE)
                    weave(og, 1)

                # ===== L2 full =====
                psa = pp.tile([128, 2, 512], F32, tag="ps", name=f"ps2fa_{t}")
                for m in range(2):
                    for k in range(4):
                        nc.tensor.matmul(psa[:, m, :],
                                         w2f[:, k, 128 * m:128 * m + 128],
                                         a1f[:, k, :],
                                         start=(k == 0), stop=(k == 3))
                psb = pp.tile([128, 512], F32, tag="ps", name=f"ps2fb_{t}")
                for k in range(4):
                    nc.tensor.matmul(psb[:], w2f[:, k, 256:384], a1f[:, k, :],
                                     start=(k == 0), stop=(k == 3))
                for m in range(2):
                    nc.scalar.activation(a2[:, m, :], psa[:, m, :], LRELU,
                                         bias=b2f[:, m:m + 1], alpha=SLOPE)
                nc.scalar.activation(a2[:, 2, :], psb[:], LRELU,
                                     bias=b2f[:, 2:3], alpha=SLOPE)
                nc.vector.memset(a2[64:65, 2, :], 1.0)
                weave(og, 1)

                # ===== L2 hori/vert (pairs of slice-pairs) =====
                for g in range(3):
                    rr = [2 * g] if g == 2 else [2 * g, 2 * g + 1]
                    ps = pp.tile([128, len(rr), 512], F32, tag="ps",
                                 name=f"ps2hv_{t}_{g}")
                    for i, r in enumerate(rr):
                        for kk in range(2):
                            nc.tensor.matmul(ps[0:64, i, :], w2h[:, kk, :],
                                             a1h[:, r, kk, :],
                                             start=(kk == 0), stop=False,
                                             skip_group_check=True)
                            nc.tensor.matmul(ps[64:128, i, :], w2v[:, kk, :],
                                             a1v[:, r, kk, :],
                                             start=(kk == 0), stop=False,
                                             skip_group_check=True)
                        nc.tensor.matmul(ps[0:128, i, :], w2r[:],
                                         a1r[0:64, r, :],
                                         start=False, stop=True,
                                         skip_group_check=True)
                    nc.scalar.activation(a2[:, 3 + 2 * g:3 + 2 * g + len(rr), :],
                                         ps[:], LRELU, bias=b2hv[:, 0:1],
                                         alpha=SLOPE)
                    weave(og, 1)

                weave(og, 8)
                prev = (a2, t)

            og = out_groups(*prev)
            weave(og, 9)

    nc.compile()
    return nc


_NC_CACHE = None


def _get_nc():
    global _NC_CACHE
    if _NC_CACHE is None:
        _NC_CACHE = _build_nc()
    return _NC_CACHE


def _prep_weights(inputs):
    W_df = np.asarray(inputs["W_df"], dtype=np.float32)
    b_df = np.asarray(inputs["b_df"], dtype=np.float32)
    W_pf = np.asarray(inputs["W_pf"], dtype=np.float32)
    b_pf = np.asarray(inputs["b_pf"], dtype=np.float32)
    W_dh = np.asarray(inputs["W_dh"], dtype=np.float32)
    b_dh = np.asarray(inputs["b_dh"], dtype=np.float32)
    W_ph = np.asarray(inputs["W_ph"], dtype=np.float32)
    b_ph = np.asarray(inputs["b_ph"], dtype=np.float32)
    W_dv = np.asarray(inputs["W_dv"], dtype=np.float32)
    b_dv = np.asarray(inputs["b_dv"], dtype=np.float32)
    W_pv = np.asarray(inputs["W_pv"], dtype=np.float32)
    b_pv = np.asarray(inputs["b_pv"], dtype=np.float32)
    W_out = np.asarray(inputs["W_out"], dtype=np.float32)
    b_out = np.asarray(inputs["b_out"], dtype=np.float32)

    # --- L1 full: block-diag per 128-chunk, bias folded in pad row 125 ---
    W1F = np.zeros((128, 4, 128), np.float32)
    for c in range(17):
        k, lc = c // 5, c % 5
        W1F[25 * lc:25 * lc + 25, k, 25 * lc:25 * lc + 25] = W_df[c].T  # [l,m]
        W1F[125, k, 25 * lc:25 * lc + 25] = b_df[25 * c:25 * c + 25]
    w1f = W1F.astype(BF).reshape(128, 512)

    # --- L1 h/v mains: [86 rows (5c+j | ones), 2 x 128 cols (16c+m)] ---
    def l1hv(Wd, b1):
        A = np.zeros((86, 256), np.float32)
        for c in range(16):
            A[5 * c:5 * c + 5, 16 * c:16 * c + 16] = Wd[c].T  # [j, m]
        A[85, :] = b1[0:256]
        return A.astype(BF).reshape(86, 256)

    w1h = l1hv(W_dh, b_dh)
    w1v = l1hv(W_dv, b_dv)

    # --- merged h+v runt (class 16) as fp8 DoubleRow planes ---
    W1R = np.zeros((86, 2, 64), np.float32)
    W1R[80:85, 0, 0:16] = W_dh[16].T
    W1R[85, 0, 0:16] = b_dh[256:272]
    W1R[80:85, 1, 32:48] = W_dv[16].T
    W1R[85, 1, 32:48] = b_dv[256:272]
    w1r = (SW1R * W1R).astype(E4).reshape(86, 128)

    # --- L2 full: K = padded class-major feature rows, M = 320(+64 pad) ---
    W2F = np.zeros((128, 4, 384), np.float32)
    for k in range(4):
        for p in range(125):
            c, m = 5 * k + p // 25, p % 25
            if c < 17:
                W2F[p, k, 0:320] = W_pf[:, 25 * c + m]
    w2f = W2F.astype(BF).reshape(128, 1536)

    # --- L2 h/v chunks + merged runt (K=64 with zero gaps) ---
    W2H = np.zeros((128, 2, 64), np.float32)
    W2V = np.zeros((128, 2, 64), np.float32)
    for kk in range(2):
        W2H[:, kk, :] = W_ph[:, 128 * kk:128 * kk + 128].T
        W2V[:, kk, :] = W_pv[:, 128 * kk:128 * kk + 128].T
    w2h = W2H.astype(BF).reshape(128, 128)
    w2v = W2V.astype(BF).reshape(128, 128)
    W2R = np.zeros((64, 128), np.float32)
    W2R[0:16, 0:64] = W_ph[:, 256:272].T
    W2R[32:48, 64:128] = W_pv[:, 256:272].T
    w2r = W2R.astype(BF)

    # --- out layer: a2 slot map -> W_out rows ---
    W3 = np.zeros((128, 8, 1600), np.float32)
    Wo = W_out  # [1600, 64, 15]
    for q in range(3):
        for p in range(128):
            f = 128 * q + p
            if f < 320:
                W3[p, q, :] = Wo[:, f // 5, f % 5]
    W3[64, 2, :] = b_out  # ones slot carries the output bias
    for r in range(5):
        for p in range(128):
            if p < 64:
                W3[p, 3 + r, :] = Wo[:, p, 5 + r]
            else:
                W3[p, 3 + r, :] = Wo[:, p - 64, 10 + r]
    w3 = W3.astype(BF).reshape(128, 12800)

    b2f = np.zeros((128, 3), np.float32)
    for m in range(3):
        n = min(128, 320 - 128 * m)
        b2f[0:n, m] = b_pf[128 * m:128 * m + n]
    b2hv = np.zeros((128, 1), np.float32)
    b2hv[0:64, 0] = b_ph
    b2hv[64:128, 0] = b_pv

    return {
        "w1f": w1f, "w1h": w1h, "w1v": w1v, "w1r": w1r,
        "w2f": w2f, "w2h": w2h, "w2v": w2v, "w2r": w2r,
        "w3": w3, "b2f": b2f, "b2hv": b2hv,
    }


def _prep_onehot(xs):
    """xs [BC, 25] int -> (ohf [128, NTILES*2048], ohhv [86, NTILES*5120])
    as fp8 bytes; h/v slices interleaved; ones pad rows carry biases."""
    ONE = np.float32(1.0).astype(E4).view(np.uint8)
    bidx = np.arange(BC)[:, None]
    ll = np.arange(25)[None, :]

    ohp = np.zeros((BC, 512), np.uint8)
    cols = 128 * (xs // 5) + 25 * (xs % 5) + ll
    ohp[bidx, cols] = ONE
    for k in range(4):
        ohp[:, 128 * k + 125] = ONE
    ohf = np.ascontiguousarray(
        ohp.T.reshape(4, 128, NTILES, 512).transpose(1, 2, 0, 3)
    ).reshape(128, NTILES * 2048)

    def hv(sl, rows):
        o = np.zeros((BC, 5, 86), np.uint8)
        o[bidx, sl, rows] = ONE
        o[:, :, 85] = ONE
        return o

    oh = hv(np.broadcast_to(ll // 5, xs.shape), 5 * xs + (ll % 5))
    ov = hv(np.broadcast_to(ll % 5, xs.shape), 5 * xs + (ll // 5))
    both = np.stack([oh, ov], axis=2)          # [BC, 5, 2, 86]
    t = both.transpose(3, 1, 2, 0).reshape(86, 5, 2, NTILES, 512)
    ohhv = np.ascontiguousarray(
        t.transpose(0, 3, 1, 2, 4)).reshape(86, NTILES * 5120)
    return ohf.view(E4).copy(), ohhv.view(E4).copy()


def kernel(**inputs) -> np.ndarray:
    x = np.asarray(inputs["x"]).astype(np.int64)
    assert x.shape == (B_FULL, 25), x.shape

    shared = _prep_weights(inputs)
    nc = _get_nc()

    in_maps = []
    for core in range(NCORES):
        xs = x[core * BC:(core + 1) * BC]
        ohf, ohhv = _prep_onehot(xs)
        m = dict(shared)
        m["ohf"] = ohf
        m["ohhv"] = ohhv
        in_maps.append(m)

    res = run_bass_kernel_spmd(nc, in_maps, core_ids=list(range(NCORES)))
    global LAST_RESULTS
    LAST_RESULTS = res
    out = np.concatenate([res.results[i]["y"].astype(np.float32)
                          for i in range(NCORES)], axis=0)
    return out


LAST_RESULTS = None
